# revision 14
# baseline (speedup 1.0000x reference)
"""Trainium2 Bass kernel for nn_CausalMultiHypothesisGraphTransformerLayer.

Sharding: 8 cores = (batch b in 0..3) x (head-group hg in 0..1).
Each core runs the full trunk for its batch (duplicated across the head-group
pair -- cheap) and the GATv2 edge phase for its 4 of 8 heads across all 3
hypotheses.  The host sums the two half-head partials, applies the head
mean + bias, and reassembles full outputs.

Graph handling: the reference graph is the 48x48 4-neighbour grid, so all
segment/gather ops become free-axis access-pattern shifts on a [features, r*c]
layout -- no gathers on device.  If the incoming edge_index is not exactly the
grid, a numpy fallback computes the reference math directly.

GATv2 logit decomposition: att_d * leaky_relu(u_d) == sign(att_d) *
leaky_relu(|att_d| * u_d), so |att| is folded into Wl/Wr on the host and the
per-edge logit becomes a +-1-weighted partition reduction (a PE matmul) of
Prelu(xl_shifted + xr).  Softmax uses unnormalised exp (no max subtraction;
logits are O(1)) with invalid-edge positions zeroed.
"""

import sys
if "/opt/trn_rl_repo" not in sys.path:
    sys.path.insert(0, "/opt/trn_rl_repo")

import numpy as np

R = 48
C = 48
N = R * C          # 2304
E = 256
E3 = 3 * E         # 768
HID = 128
FF = 512
NHYP = 3
HEADS = 8
B = 4
HPC = 4            # heads per core
ALPHA = 0.2
NDIR = 5

# chunking of the node axis for matmuls (PSUM bank = 512 fp32)
CHUNKS = [(0, 512), (512, 512), (1024, 512), (1536, 512), (2048, 256)]

# directions: (name, shift s such that src = dst + s, rim spec for invalid dst)
DIRS = [
    ("self", 0, None),
    ("dl", 1, ("c", C - 1)),
    ("dr", -1, ("c", 0)),
    ("du", C, ("r", R - 1)),
    ("dd", -C, ("r", 0)),
]

_CACHE = {}


def _grid_edge_set():
    idx = np.arange(N).reshape(R, C)
    pairs = [(idx[1:, :], idx[:-1, :]), (idx[:-1, :], idx[1:, :]),
             (idx[:, 1:], idx[:, :-1]), (idx[:, :-1], idx[:, 1:])]
    src = np.concatenate([a.ravel() for a, _ in pairs])
    dst = np.concatenate([b.ravel() for _, b in pairs])
    return np.stack([src, dst]).astype(np.int64)


def _is_grid(edge_index):
    e = np.asarray(edge_index, dtype=np.int64)
    g = _grid_edge_set()
    if e.shape != g.shape:
        return False
    eo = e[:, np.lexsort((e[1], e[0]))]
    go = g[:, np.lexsort((g[1], g[0]))]
    return bool(np.array_equal(eo, go))


# ---------------------------------------------------------------------------
# numpy fallback (exact reference math, any graph)
# ---------------------------------------------------------------------------

def _np_forward(x1, x2, x3, edge_index, p):
    f32 = np.float32
    src, dst = edge_index[0].astype(np.int64), edge_index[1].astype(np.int64)

    def seg_sum(vals, idx, n):
        out = np.zeros((n,) + vals.shape[1:], vals.dtype)
        np.add.at(out, idx, vals)
        return out

    def seg_max(vals, idx, n):
        out = np.full((n,) + vals.shape[1:], -np.inf, vals.dtype)
        np.maximum.at(out, idx, vals)
        return out

    def ln(x, g, b):
        m = x.mean(-1, keepdims=True)
        v = ((x - m) ** 2).mean(-1, keepdims=True)
        return (x - m) / np.sqrt(v + 1e-5) * g + b

    deg = seg_sum(np.ones(src.shape[0], f32), dst, N) + 1.0
    dinv = 1.0 / np.sqrt(deg)
    gnorm = dinv[src] * dinv[dst]

    def gcn(x, W, b):
        h = x @ W
        agg = seg_sum(h[src] * gnorm[:, None, None], dst, N)
        agg = agg + h * (1.0 / deg)[:, None, None]
        return agg + b

    loop = np.arange(N, dtype=np.int64)
    s2 = np.concatenate([src, loop])
    d2 = np.concatenate([dst, loop])

    def gatv2(x, Wl, Wr, att, b):
        bsz = x.shape[1]
        xl = (x @ Wl).reshape(N, bsz, HEADS, E)
        xr = (x @ Wr).reshape(N, bsz, HEADS, E)
        e = xl[s2] + xr[d2]
        e = np.where(e > 0, e, ALPHA * e)
        logit = np.einsum('ebhd,hd->ebh', e, att)
        m = seg_max(logit, d2, N)
        ex = np.exp(logit - m[d2])
        den = seg_sum(ex, d2, N)
        alpha = ex / den[d2]
        out = seg_sum(alpha[..., None] * xl[s2], d2, N)
        return out.mean(axis=2) + b

    x = np.concatenate([x1, x2, x3], axis=1)
    bsz = x.shape[0]
    xs = x.reshape(bsz, E3, N).transpose(2, 0, 1)
    node = xs @ p['proj_W'] + p['proj_b']

    z = np.maximum(node @ p['mg_W1'] + p['mg_b1'], 0) @ p['mg_W2'] + p['mg_b2']
    sig = 1.0 / (1.0 + np.exp(-z))
    x_conf = sig * node
    x_adj = (1.0 - sig) * node

    adj = ln(gcn(x_adj, p['gcnA_W'], p['gcnA_b']), p['ln0_g'], p['ln0_b'])
    conf = ln(gcn(x_conf, p['gcnC_W'], p['gcnC_b']), p['ln1_g'], p['ln1_b'])

    hyps = []
    for i in range(NHYP):
        c = p['civ'][i]
        iv = np.maximum(conf @ c['W1'] + c['b1'], 0) @ c['W2'] + c['b2']
        feat = adj + conf + iv
        g = p['gat'][i]
        hyps.append(gatv2(feat, g['Wl'], g['Wr'], g['att'], g['b']))

    orig = adj + conf
    ffn = np.maximum(orig @ p['ffn_W1'] + p['ffn_b1'], 0) @ p['ffn_W2'] + p['ffn_b2']
    orig = ln(orig + ffn, p['ln2_g'], p['ln2_b'])

    outs = [h.transpose(1, 2, 0).reshape(bsz, E, R, C).astype(f32) for h in hyps]
    outs.append(orig.transpose(1, 2, 0).reshape(bsz, E, R, C).astype(f32))
    return tuple(outs)


# ---------------------------------------------------------------------------
# device program
# ---------------------------------------------------------------------------

def _build_program():
    import concourse.bass as bass  # noqa: F401
    from concourse import bacc
    import concourse.mybir as mybir
    import concourse.tile as tile
    from contextlib import ExitStack

    F32 = mybir.dt.float32
    F16 = mybir.dt.float16
    AF = mybir.ActivationFunctionType
    nc = bacc.Bacc(None, target_bir_lowering=False)

    def dram(name, shape, dtype=F32, kind="ExternalInput"):
        return nc.dram_tensor(name, list(shape), dtype, kind=kind)

    # ---- inputs
    xs16 = dram("xs16", [128, 6, N], F16)
    dinvb16 = dram("dinvb16", [128, N], F16)
    wp16 = dram("wp16", [128, 6, E], F16)
    bp = dram("bp", [128, 2])
    mgw1 = dram("mgw1", [128, 2, HID], F16)
    mgb1 = dram("mgb1", [128, 1])
    mgw2 = dram("mgw2", [128, E], F16)
    mgb2 = dram("mgb2", [128, 2])
    gaw = dram("gaw", [128, 2, E], F16)
    gab = dram("gab", [128, 2])
    gcw = dram("gcw", [128, 2, E], F16)
    gcb = dram("gcb", [128, 2])
    lng = [dram(f"ln{i}g", [128, 2]) for i in range(3)]
    lnb = [dram(f"ln{i}b", [128, 2]) for i in range(3)]
    civw1 = [dram(f"civw1_{i}", [128, 2, HID], F16) for i in range(NHYP)]
    civb1 = [dram(f"civb1_{i}", [128, 1]) for i in range(NHYP)]
    civw2 = [dram(f"civw2_{i}", [128, E], F16) for i in range(NHYP)]
    civb2 = [dram(f"civb2_{i}", [128, 2]) for i in range(NHYP)]
    ffnw1 = dram("ffnw1", [128, 2, FF], F16)
    ffnb1 = dram("ffnb1", [128, 4])
    ffnw2 = dram("ffnw2", [128, 4, E], F16)
    ffnb2 = dram("ffnb2", [128, 2])
    wl16 = [dram(f"wl16_{i}", [128, 2, HPC * E], F16) for i in range(NHYP)]
    wr16 = [dram(f"wr16_{i}", [128, 2, HPC * E], F16) for i in range(NHYP)]
    sgn16 = [dram(f"sgn16_{i}", [128, 8, 2], F16) for i in range(NHYP)]
    eye2 = dram("eye2", [2, 2], F16)
    ones1h = dram("ones1h", [1, 128], F16)
    onesov16 = dram("onesov16", [128, 2, 1], F16)
    eps1 = dram("eps1", [1, 1])

    # ---- outputs
    hyp_out = [dram(f"hyp{i}", [128, 2, N], F16, kind="ExternalOutput")
               for i in range(NHYP)]
    orig_out = dram("orig", [128, 2, N], kind="ExternalOutput")

    with ExitStack() as stack:
        tc = stack.enter_context(tile.TileContext(nc))
        const = stack.enter_context(tc.tile_pool(name="const", bufs=1))
        live = stack.enter_context(tc.tile_pool(name="live", bufs=1))
        psum = stack.enter_context(tc.tile_pool(name="psum", bufs=2, space="PSUM"))
        lgpool = stack.enter_context(tc.tile_pool(name="lgp", bufs=1, space="PSUM"))
        dpool = stack.enter_context(tc.tile_pool(name="dpool", bufs=2, space="DRAM"))

        def load(pool, dr, tag=None, bufs=None):
            kw = {}
            if bufs is not None:
                kw["bufs"] = bufs
            t = pool.tile(list(dr.shape), dr.dtype, tag=tag or dr.name,
                          name=dr.name, **kw)
            nc.sync.dma_start(out=t[:], in_=dr[:])
            return t

        t_wp = load(const, wp16)
        t_bp = load(const, bp)
        t_mgw1 = load(const, mgw1)
        t_mgb1 = load(const, mgb1)
        t_mgw2 = load(const, mgw2)
        t_mgb2 = load(const, mgb2)
        t_gaw = load(const, gaw)
        t_gab = load(const, gab)
        t_gcw = load(const, gcw)
        t_gcb = load(const, gcb)
        t_lng = [load(const, x) for x in lng]
        t_lnb = [load(const, x) for x in lnb]
        t_civw1 = [load(const, x) for x in civw1]
        t_civb1 = [load(const, x) for x in civb1]
        t_civw2 = [load(const, x) for x in civw2]
        t_civb2 = [load(const, x) for x in civb2]
        t_ffnw1 = load(const, ffnw1)
        t_ffnb1 = load(const, ffnb1)
        t_ffnw2 = load(const, ffnw2)
        t_ffnb2 = load(const, ffnb2)
        t_sgn = [load(const, x) for x in sgn16]
        t_eye2 = load(const, eye2)
        t_ones1h = load(const, ones1h)
        t_onesov = load(const, onesov16)
        t_eps1 = load(const, eps1)
        t_dinvb = load(const, dinvb16)

        MM = nc.tensor.matmul

        def mm_to(out_write, lhsT_fn, ktiles, m, rhs_fn, func=AF.Copy,
                  bias=None, scale=None):
            for (c0, w) in CHUNKS:
                ps = psum.tile([128, 512], F32, tag="mmps", name="ps")
                for kt in range(ktiles):
                    MM(ps[0:m, 0:w], lhsT_fn(kt), rhs_fn(kt, c0, w),
                       start=(kt == 0), stop=(kt == ktiles - 1))
                kw = {}
                if bias is not None:
                    kw["bias"] = bias
                if scale is not None:
                    kw["scale"] = scale
                out_write(c0, w, ps[0:m, 0:w], func, kw)

        def act_write(dst_tile, pt):
            def w(c0, w_, ps, func, kw):
                if pt is None:
                    nc.scalar.activation(dst_tile[:, c0:c0 + w_], ps, func, **kw)
                else:
                    nc.scalar.activation(dst_tile[:, pt, c0:c0 + w_], ps, func, **kw)
            return w

        # =============== trunk (scoped pools) ===============
        # e16 tag: rotating [128,2,N] f16 slots; the allocation ORDER below is
        # load-bearing for liveness with bufs=4 -- do not reorder casually.
        with tc.tile_pool(name="early", bufs=1) as early:

            def e16(name):
                return early.tile([128, 2, N], F16, tag="e16", name=name, bufs=4)

            def layer_norm(x16, gt, btt, out_tile):
                """LN over the 256 partition-feature dims of [128,2,N] f16."""
                sq16 = e16("lnsq")
                nc.scalar.activation(sq16[:], x16[:], AF.Square)
                mean = early.tile([1, N], F32, tag="lnmean", name="mean", bufs=1)
                msq = early.tile([1, N], F32, tag="lnmsq", name="msq", bufs=1)
                for (c0, w) in CHUNKS:
                    ps = psum.tile([128, 512], F32, tag="mmps", name="ps")
                    psq = psum.tile([128, 512], F32, tag="mmps", name="psq")
                    for kt in range(2):
                        MM(ps[0:1, 0:w], t_onesov[:, kt, :],
                           x16[:, kt, c0:c0 + w], start=(kt == 0), stop=(kt == 1))
                    for kt in range(2):
                        MM(psq[0:1, 0:w], t_onesov[:, kt, :],
                           sq16[:, kt, c0:c0 + w], start=(kt == 0), stop=(kt == 1))
                    nc.scalar.copy(mean[0:1, c0:c0 + w], ps[0:1, 0:w])
                    nc.scalar.copy(msq[0:1, c0:c0 + w], psq[0:1, 0:w])
                var = early.tile([1, N], F32, tag="lnvar", name="var", bufs=1)
                nc.vector.tensor_mul(out=var[:], in0=mean[:], in1=mean[:])
                nc.vector.tensor_sub(out=var[:], in0=msq[:], in1=var[:])
                nc.scalar.activation(var[:], var[:], AF.Ln, bias=t_eps1[:])
                nc.scalar.activation(var[:], var[:], AF.Exp, scale=-0.5)
                mean16 = early.tile([1, N], F16, tag="lnmean16", name="mean16", bufs=1)
                nc.vector.tensor_copy(out=mean16[:], in_=mean[:])
                rstd16 = early.tile([1, N], F16, tag="lnrstd16", name="rstd16", bufs=1)
                nc.vector.tensor_copy(out=rstd16[:], in_=var[:])
                mb = early.tile([128, N], F16, tag="lnmb", name="mb", bufs=1)
                rb = early.tile([128, N], F16, tag="lnrb", name="rb", bufs=1)
                for srct, dstt in ((mean16, mb), (rstd16, rb)):
                    for (c0, w) in CHUNKS:
                        ps = psum.tile([128, 512], F32, tag="mmps", name="ps")
                        MM(ps[:, 0:w], t_ones1h[:], srct[0:1, c0:c0 + w],
                           start=True, stop=True)
                        nc.scalar.copy(dstt[:, c0:c0 + w], ps[:, 0:w])
                tdiff = e16("lntd")
                for pt in range(2):
                    nc.vector.tensor_sub(out=tdiff[:, pt], in0=x16[:, pt],
                                         in1=mb[:])
                    nc.vector.tensor_mul(out=tdiff[:, pt], in0=tdiff[:, pt],
                                         in1=rb[:])
                    nc.scalar.activation(out_tile[:, pt], tdiff[:, pt],
                                         AF.Identity,
                                         scale=gt[:, pt:pt + 1],
                                         bias=btt[:, pt:pt + 1])
                return out_tile

            node16 = e16("node16")
            with tc.tile_pool(name="xsp", bufs=1) as xsp:
                t_xs = load(xsp, xs16)
                for pt in range(2):
                    mm_to(act_write(node16, pt),
                          lambda kt, pt=pt: t_wp[:, kt, 128 * pt:128 * pt + 128],
                          6, 128,
                          lambda kt, c0, w: t_xs[:, kt, c0:c0 + w],
                          func=AF.Identity, bias=t_bp[:, pt:pt + 1])

            z1full = e16("z1full")
            z1 = z1full[:, 0]
            def z1_write(c0, w_, ps, func, kw):
                nc.scalar.activation(z1[:, c0:c0 + w_], ps, func, **kw)
            mm_to(z1_write,
                  lambda kt: t_mgw1[:, kt, :], 2, 128,
                  lambda kt, c0, w: node16[:, kt, c0:c0 + w],
                  func=AF.Relu, bias=t_mgb1[:, 0:1])
            zm16 = e16("zm16")
            for pt in range(2):
                mm_to(act_write(zm16, pt),
                      lambda kt, pt=pt: t_mgw2[:, 128 * pt:128 * pt + 128],
                      1, 128,
                      lambda kt, c0, w: z1[:, c0:c0 + w],
                      func=AF.Identity, bias=t_mgb2[:, pt:pt + 1])
            sgm16 = e16("sgm16")
            for pt in range(2):
                nc.scalar.activation(sgm16[:, pt], zm16[:, pt], AF.Exp,
                                     scale=-1.0)
            nc.vector.tensor_scalar_add(out=sgm16[:], in0=sgm16[:], scalar1=1.0)
            with nc.allow_low_precision("sigmoid in fp16 is within tolerance"):
                nc.vector.reciprocal(out=sgm16[:], in_=sgm16[:])
            xconf16 = early.tile([128, 2, N], F16, tag="xm16", name="xconf16",
                                 bufs=2)
            nc.vector.tensor_mul(out=xconf16[:], in0=node16[:], in1=sgm16[:])
            xadj16 = early.tile([128, 2, N], F16, tag="xm16", name="xadj16",
                                bufs=2)
            nc.vector.tensor_sub(out=xadj16[:], in0=node16[:], in1=xconf16[:])

            def gcn_ln(x16, wt, bt, gt, btt, out_tile):
                h16 = e16("h16")
                for pt in range(2):
                    mm_to(act_write(h16, pt),
                          lambda kt, pt=pt: wt[:, kt, 128 * pt:128 * pt + 128],
                          2, 128,
                          lambda kt, c0, w: x16[:, kt, c0:c0 + w],
                          func=AF.Copy)
                ht = e16("ht")
                for pt in range(2):
                    nc.vector.tensor_mul(out=ht[:, pt], in0=h16[:, pt],
                                         in1=t_dinvb[:])
                S4 = e16("Sgcn")
                S = S4[:].rearrange("p t (r c) -> p t r c", r=R)
                nc.vector.tensor_copy(out=S4[:], in_=ht[:])
                hv = ht[:].rearrange("p t (r c) -> p t r c", r=R)
                for pt in range(2):
                    nc.vector.tensor_add(out=S[:, pt, :, 0:C - 1],
                                         in0=S[:, pt, :, 0:C - 1],
                                         in1=hv[:, pt, :, 1:C])
                    nc.vector.tensor_add(out=S[:, pt, :, 1:C],
                                         in0=S[:, pt, :, 1:C],
                                         in1=hv[:, pt, :, 0:C - 1])
                    nc.vector.tensor_add(out=S[:, pt, 0:R - 1, :],
                                         in0=S[:, pt, 0:R - 1, :],
                                         in1=hv[:, pt, 1:R, :])
                    nc.vector.tensor_add(out=S[:, pt, 1:R, :],
                                         in0=S[:, pt, 1:R, :],
                                         in1=hv[:, pt, 0:R - 1, :])
                biased = e16("biased")
                for pt in range(2):
                    nc.vector.tensor_mul(out=biased[:, pt], in0=S4[:, pt],
                                         in1=t_dinvb[:])
                for pt in range(2):
                    nc.scalar.activation(biased[:, pt], biased[:, pt],
                                         AF.Identity, bias=bt[:, pt:pt + 1])
                return layer_norm(biased, gt, btt, out_tile)

            adjE = early.tile([128, 2, N], F16, tag="adjE", name="adjE", bufs=1)
            gcn_ln(xadj16, t_gaw, t_gab, t_lng[0], t_lnb[0], adjE)
            conf16 = live.tile([128, 2, N], F16, tag="conf16", name="conf16")
            gcn_ln(xconf16, t_gcw, t_gcb, t_lng[1], t_lnb[1], conf16)

            ac16 = live.tile([128, 2, N], F16, tag="ac16", name="ac16")
            nc.vector.tensor_add(out=ac16[:], in0=adjE[:], in1=conf16[:])

            # FFN / orig path
            f1 = early.tile([128, 4, N], F16, tag="f1", name="f1", bufs=1)
            for mpt in range(4):
                mm_to(act_write(f1, mpt),
                      lambda kt, mpt=mpt: t_ffnw1[:, kt, 128 * mpt:128 * mpt + 128],
                      2, 128,
                      lambda kt, c0, w: ac16[:, kt, c0:c0 + w],
                      func=AF.Relu, bias=t_ffnb1[:, mpt:mpt + 1])
            f2b16 = e16("f2b16")
            for pt in range(2):
                mm_to(act_write(f2b16, pt),
                      lambda kt, pt=pt: t_ffnw2[:, kt, 128 * pt:128 * pt + 128],
                      4, 128,
                      lambda kt, c0, w: f1[:, kt, c0:c0 + w],
                      func=AF.Identity, bias=t_ffnb2[:, pt:pt + 1])
            res16 = e16("res16")
            nc.vector.tensor_add(out=res16[:], in0=ac16[:], in1=f2b16[:])
            origt = early.tile([128, 2, N], F32, tag="origt", name="origt",
                               bufs=1)
            layer_norm(res16, t_lng[2], t_lnb[2], origt)
            nc.sync.dma_start(out=orig_out[:], in_=origt[:])

        # =============== GAT hypotheses ===============
        with tc.tile_pool(name="gat", bufs=1) as gat, \
             tc.tile_pool(name="gatw", bufs=2) as gatw:
            for i in range(NHYP):
                t_wli = load(gatw, wl16[i], tag="wl", bufs=2)
                t_wri = load(gatw, wr16[i], tag="wr", bufs=2)

                # civ: iv = relu(conf@W1+b1)@W2 + b2 ; feat = ac + iv
                c1 = gat.tile([128, N], F16, tag="c1", name="c1", bufs=1)
                mm_to(act_write(c1, None),
                      lambda kt, i=i: t_civw1[i][:, kt, :], 2, 128,
                      lambda kt, c0, w: conf16[:, kt, c0:c0 + w],
                      func=AF.Relu, bias=t_civb1[i][:, 0:1])
                feat16 = gat.tile([128, 2, N], F16, tag="feat16",
                                  name="feat16", bufs=1)
                for pt in range(2):
                    for (c0, w) in CHUNKS:
                        ps = psum.tile([128, 512], F32, tag="mmps", name="ps")
                        MM(ps[:, 0:w],
                           t_civw2[i][:, 128 * pt:128 * pt + 128],
                           c1[:, c0:c0 + w], start=True, stop=True)
                        ivc = gat.tile([128, 512], F16, tag="ivc",
                                       name="ivc", bufs=2)
                        nc.scalar.activation(ivc[:, 0:w], ps[:, 0:w],
                                             AF.Identity,
                                             bias=t_civb2[i][:, pt:pt + 1])
                        nc.vector.tensor_add(out=feat16[:, pt, c0:c0 + w],
                                             in0=ivc[:, 0:w],
                                             in1=ac16[:, pt, c0:c0 + w])

                hsum = gat.tile([128, 2, N], F16, tag="hsum", name="hsum",
                                bufs=1)
                for q in range(2):  # head-pair halves
                    xl = gat.tile([128, 4, N], F16, tag="xl", name="xl", bufs=1)
                    xr = gat.tile([128, 4, N], F16, tag="xr", name="xr", bufs=1)
                    for dst_t, wmat in ((xl, t_wli), (xr, t_wri)):
                        for j in range(4):
                            col = 512 * q + 128 * j
                            mm_to(act_write(dst_t, j),
                                  lambda kt, col=col, wmat=wmat:
                                      wmat[:, kt, col:col + 128],
                                  2, 128,
                                  lambda kt, c0, w: feat16[:, kt, c0:c0 + w],
                                  func=AF.Copy)

                    # logits per direction -> exp -> E16 per dir
                    e16d = []
                    for d, (dname, s, rim) in enumerate(DIRS):
                        lgps = lgpool.tile([2, 5, 512], F32, tag="lgps",
                                           name="lgps")
                        for j in range(4):
                            zt = gat.tile([128, N], F16, tag="zt", name="zt",
                                          bufs=1)
                            if s == 0:
                                nc.vector.tensor_add(out=zt[:], in0=xl[:, j],
                                                     in1=xr[:, j])
                            elif s > 0:
                                nc.vector.tensor_add(out=zt[:, 0:N - s],
                                                     in0=xl[:, j, s:N],
                                                     in1=xr[:, j, 0:N - s])
                                nc.gpsimd.memset(zt[:, N - s:N], 0.0)
                            else:
                                o = -s
                                nc.vector.tensor_add(out=zt[:, o:N],
                                                     in0=xl[:, j, 0:N - o],
                                                     in1=xr[:, j, o:N])
                                nc.gpsimd.memset(zt[:, 0:o], 0.0)
                            rt = gat.tile([128, N], F16, tag="rt", name="rt",
                                          bufs=2)
                            nc.scalar.activation(rt[:], zt[:], AF.Prelu,
                                                 alpha=ALPHA)
                            for ci, (c0, w) in enumerate(CHUNKS):
                                MM(lgps[:, ci, 0:w],
                                   t_sgn[i][:, 4 * q + j, :],
                                   rt[:, c0:c0 + w],
                                   start=(j == 0), stop=(j == 3))
                        ed = gat.tile([2, 5 * 512], F16, tag=f"e16_{d}",
                                      name="ed", bufs=1)
                        nc.scalar.activation(
                            ed[:], lgps[:].rearrange("p a b -> p (a b)"), AF.Exp)
                        if rim is not None:
                            ax, idx = rim
                            edv = ed[:, 0:N].rearrange("p (r c) -> p r c", r=R)
                            if ax == "c":
                                nc.gpsimd.memset(edv[:, :, idx:idx + 1], 0.0)
                            else:
                                nc.gpsimd.memset(edv[:, idx:idx + 1, :], 0.0)
                        e16d.append(ed)

                    # den = sum over dirs (PE accumulate), gsc16 = 1/den
                    dps = lgpool.tile([2, 5, 512], F32, tag="lgps", name="dps")
                    for d in range(NDIR):
                        for ci, (c0, w) in enumerate(CHUNKS):
                            MM(dps[:, ci, 0:w], t_eye2[:], e16d[d][:, c0:c0 + w],
                               start=(d == 0), stop=(d == NDIR - 1))
                    gsc16 = gat.tile([2, N], F16, tag="gsc16", name="gsc16",
                                     bufs=1)
                    with nc.allow_low_precision("softmax scale fp16 ok"):
                        for ci, (c0, w) in enumerate(CHUNKS):
                            nc.vector.reciprocal(out=gsc16[:, c0:c0 + w],
                                                 in_=dps[:, ci, 0:w])

                    # gE = E*g (in place), staged to DRAM for the broadcast
                    est = dpool.tile([10, N], F16, tag="est", name="est")
                    for d in range(NDIR):
                        nc.vector.tensor_mul(out=e16d[d][:, 0:N],
                                             in0=e16d[d][:, 0:N], in1=gsc16[:])
                        nc.sync.dma_start(out=est[2 * d:2 * d + 2, :],
                                          in_=e16d[d][:, 0:N])

                    # weighted sum over (head, direction)
                    for hq in range(2):
                        acc = gat.tile([128, 2, N], F16, tag="acc", name="acc",
                                       bufs=1)
                        for d, (dname, s, rim) in enumerate(DIRS):
                            ab = gat.tile([128, N], F16, tag="ab", name="ab",
                                          bufs=2)
                            row = 2 * d + hq
                            nc.sync.dma_start(
                                out=ab[:],
                                in_=est[row:row + 1, :].broadcast_to((128, N)))
                            for pt in range(2):
                                hp = 2 * hq + pt
                                if s == 0:
                                    nc.vector.tensor_mul(out=acc[:, pt],
                                                         in0=ab[:],
                                                         in1=xl[:, hp])
                                elif s > 0:
                                    tp = gat.tile([128, N], F16, tag="tp",
                                                  name="tp", bufs=1)
                                    nc.vector.tensor_mul(out=tp[:, 0:N - s],
                                                         in0=ab[:, 0:N - s],
                                                         in1=xl[:, hp, s:N])
                                    nc.vector.tensor_add(
                                        out=acc[:, pt, 0:N - s],
                                        in0=acc[:, pt, 0:N - s],
                                        in1=tp[:, 0:N - s])
                                else:
                                    o = -s
                                    tp = gat.tile([128, N], F16, tag="tp",
                                                  name="tp", bufs=1)
                                    nc.vector.tensor_mul(out=tp[:, o:N],
                                                         in0=ab[:, o:N],
                                                         in1=xl[:, hp, 0:N - o])
                                    nc.vector.tensor_add(out=acc[:, pt, o:N],
                                                         in0=acc[:, pt, o:N],
                                                         in1=tp[:, o:N])
                        if q == 0 and hq == 0:
                            nc.vector.tensor_copy(out=hsum[:], in_=acc[:])
                        else:
                            nc.vector.tensor_add(out=hsum[:], in0=hsum[:],
                                                 in1=acc[:])
                nc.sync.dma_start(out=hyp_out[i][:], in_=hsum[:])

    nc.finalize()
    return nc


def _get_program():
    if "nc" not in _CACHE:
        _CACHE["nc"] = _build_program()
    return _CACHE["nc"]


# ---------------------------------------------------------------------------
# host side
# ---------------------------------------------------------------------------

def _prep_inputs(x1, x2, x3, edge_index, p):
    f32, f16 = np.float32, np.float16

    def kmaj(w, ktiles):
        w = np.asarray(w, f32)
        k, m = w.shape
        assert k == ktiles * 128
        return np.ascontiguousarray(
            w.reshape(ktiles, 128, m).transpose(1, 0, 2)).astype(f16)

    def pvec(v, parts):
        v = np.asarray(v, f32)
        return np.ascontiguousarray(v.reshape(parts, 128).T).astype(f32)

    shared = {
        "wp16": kmaj(p["proj_W"], 6),
        "bp": pvec(p["proj_b"], 2),
        "mgw1": kmaj(p["mg_W1"], 2),
        "mgb1": pvec(p["mg_b1"], 1),
        "mgw2": kmaj(p["mg_W2"], 1)[:, 0, :],
        "mgb2": pvec(p["mg_b2"], 2),
        "gaw": kmaj(p["gcnA_W"], 2),
        "gab": pvec(p["gcnA_b"], 2),
        "gcw": kmaj(p["gcnC_W"], 2),
        "gcb": pvec(p["gcnC_b"], 2),
        "ffnw1": kmaj(p["ffn_W1"], 2),
        "ffnb1": pvec(p["ffn_b1"], 4),
        "ffnw2": kmaj(p["ffn_W2"], 4),
        "ffnb2": pvec(p["ffn_b2"], 2),
        "eps1": np.full((1, 1), 1e-5, f32),
        "ones1h": np.ones((1, 128), f16),
        "onesov16": np.full((128, 2, 1), 1.0 / 256.0, f16),
        "eye2": np.eye(2, dtype=f16),
    }
    for j in range(3):
        shared[f"ln{j}g"] = pvec(p[f"ln{j}_g"], 2)
        shared[f"ln{j}b"] = pvec(p[f"ln{j}_b"], 2)
    for i in range(NHYP):
        c = p["civ"][i]
        shared[f"civw1_{i}"] = kmaj(c["W1"], 2)
        shared[f"civb1_{i}"] = pvec(c["b1"], 1)
        shared[f"civw2_{i}"] = kmaj(c["W2"], 1)[:, 0, :]
        shared[f"civb2_{i}"] = pvec(c["b2"], 2)

    ei = np.asarray(edge_index, np.int64)
    deg = np.bincount(ei[1], minlength=N).astype(f32) + 1.0
    dinv = (1.0 / np.sqrt(deg)).astype(f32)
    shared["dinvb16"] = np.broadcast_to(
        dinv.astype(f16)[None, :], (128, N)).copy()

    x = np.concatenate([np.asarray(x1, f32), np.asarray(x2, f32),
                        np.asarray(x3, f32)], axis=1)  # [B, 768, R, C]
    xs_all = x.reshape(B, E3, N)

    in_maps = []
    for core in range(8):
        b, hg = core // 2, core % 2
        m = dict(shared)
        m["xs16"] = np.ascontiguousarray(
            xs_all[b].reshape(6, 128, N).transpose(1, 0, 2)).astype(f16)
        for i in range(NHYP):
            g = p["gat"][i]
            Wl = np.asarray(g["Wl"], f32)
            Wr = np.asarray(g["Wr"], f32)
            att = np.asarray(g["att"], f32)
            cols = slice((hg * HPC) * E, (hg * HPC + HPC) * E)
            m[f"wl16_{i}"] = kmaj(Wl[:, cols], 2)
            m[f"wr16_{i}"] = kmaj(Wr[:, cols], 2)
            # reduction weights: att values for this core's heads
            attc = att[hg * HPC: hg * HPC + HPC].astype(f32)
            sg = np.zeros((128, 8, 2), f16)
            for qj in range(8):
                q, j = qj // 4, qj % 4
                hprime, half = j // 2, j % 2
                hloc = 2 * q + hprime
                sg[:, qj, hprime] = attc[hloc, 128 * half:128 * half + 128]
            m[f"sgn16_{i}"] = sg
        in_maps.append(m)
    return in_maps


def kernel(x1, x2, x3, edge_index, params):
    x1 = np.asarray(x1, np.float32)
    x2 = np.asarray(x2, np.float32)
    x3 = np.asarray(x3, np.float32)
    edge_index = np.asarray(edge_index)
    p = {k: (v if isinstance(v, (list, dict)) else np.asarray(v, np.float32))
         for k, v in params.items()}
    p["civ"] = [{k2: np.asarray(v2, np.float32) for k2, v2 in c.items()}
                for c in params["civ"]]
    p["gat"] = [{k2: np.asarray(v2, np.float32) for k2, v2 in g.items()}
                for g in params["gat"]]

    if not _is_grid(edge_index):
        return _np_forward(x1, x2, x3, edge_index, p)

    from concourse.bass_utils import run_bass_kernel_spmd
    nc = _get_program()
    in_maps = _prep_inputs(x1, x2, x3, edge_index, p)
    res = run_bass_kernel_spmd(nc, in_maps, core_ids=list(range(8)))
    outs = res.results

    hyps = []
    for i in range(NHYP):
        full = np.zeros((B, E, N), np.float32)
        gb = np.asarray(params["gat"][i]["b"], np.float32)
        for b in range(B):
            part = outs[2 * b][f"hyp{i}"].astype(np.float32) + \
                outs[2 * b + 1][f"hyp{i}"].astype(np.float32)
            part = part.transpose(1, 0, 2).reshape(E, N)
            full[b] = part / float(HEADS) + gb[:, None]
        hyps.append(full.reshape(B, E, R, C))
    orig = np.zeros((B, E, N), np.float32)
    for b in range(B):
        o = outs[2 * b]["orig"]
        orig[b] = o.transpose(1, 0, 2).reshape(E, N)
    return tuple(hyps + [orig.reshape(B, E, R, C)])
